# revision 10
# baseline (speedup 1.0000x reference)
"""Trainium2 Bass kernel for a 3-layer LIF spiking net (25-step temporal scan).

Strategy (pure data parallel over batch, 8 cores):
  - Host transposes/shards x, precomputes all per-step scaled weight matrices.
  - Device (per core, B_local=8192): fc1 = w1@xT+b1 once; then a 25-step scan
    with membranes kept PSUM-resident in a scaled basis n_t = beta^(-t) * m_t,
    so the per-step decay becomes pure PSUM accumulation:
      n1 += beta^-t * cur1                      (PE, diag lhsT)
      spk1 = (n1 > beta^-t)                     (DVE is_gt, {0,1})
      n2 += beta^-t * (w2 @ spk1 + b2)          (PE)
      n1 -= beta^-(t+1) * spk1                  (PE, diag lhsT)
      spk2s = sign(n2 - beta^-t)                (ACT Sign, {-1,+1})
      n2 -= 0.5*beta^-(t+1) * (spk2s + 1)       (PE)
      cur3_t = 0.5*w3 @ spk2s + (b3 + 0.5*w3@1) (PE, natural units, packed into
                                                 32-aligned PSUM partition slots)
  - cur3 [25, 2, B] goes back to host; the tiny layer-3 LIF scan runs in numpy
    (exact fp32 semantics), producing spk3_rec / mem3_rec.
"""

import numpy as np

BETA = 0.9
T = 25
TH = 1.0
B, D, H, O = 65536, 784, 100, 2
NCORES = 8
BL = B // NCORES          # 8192 per core
BC = 1024                 # scan column chunk
NCHUNK = BL // BC         # 8
NSL = BC // 512           # 512-col sub-matmuls per chunk
DCH = 112                 # fc1 contraction chunk (784 = 7*112)
NK = D // DCH             # 7
FH = BL // 2              # fc1 column half (4096)

_CACHE = {}


def _binv(t):
    return np.float32(np.float64(BETA) ** (-t))


def _build_program():
    import concourse.bass as bass
    import concourse.mybir as mybir
    from concourse.tile import TileContext

    f32 = mybir.dt.float32
    AF = mybir.ActivationFunctionType
    ALU = mybir.AluOpType

    nc = bass.Bass()

    xT = nc.dram_tensor("xT", [D, BL], f32, kind="ExternalInput")
    w1p = nc.dram_tensor("w1p", [DCH, NK * H], f32, kind="ExternalInput")
    b1c = nc.dram_tensor("b1c", [H, 1], f32, kind="ExternalInput")
    ADh = nc.dram_tensor("ADh", [H, T * H], f32, kind="ExternalInput")
    D1h = nc.dram_tensor("D1h", [H, T * H], f32, kind="ExternalInput")
    W2h = nc.dram_tensor("W2h", [H + 1, T * H], f32, kind="ExternalInput")
    D2h = nc.dram_tensor("D2h", [H + 1, T * H], f32, kind="ExternalInput")
    W3h = nc.dram_tensor("W3h", [H + 1, T * O], f32, kind="ExternalInput")
    NTh = nc.dram_tensor("NTh", [H, T], f32, kind="ExternalInput")
    ONEh = nc.dram_tensor("ONEh", [1, BC], f32, kind="ExternalInput")
    cur3 = nc.dram_tensor("cur3", [T, O, BL], f32, kind="ExternalOutput")

    with TileContext(nc) as tc:
        with tc.tile_pool(name="wpool", bufs=1) as wp:
            w1sb = wp.tile([DCH, NK * H], f32)
            nc.sync.dma_start(out=w1sb[:], in_=w1p[:])
            b1sb = wp.tile([H, 1], f32)
            nc.sync.dma_start(out=b1sb[:], in_=b1c[:])
            ADsb = wp.tile([H, T * H], f32)
            nc.sync.dma_start(out=ADsb[:], in_=ADh[:])
            D1sb = wp.tile([H, T * H], f32)
            nc.sync.dma_start(out=D1sb[:], in_=D1h[:])
            W2sb = wp.tile([H + 1, T * H], f32)
            nc.sync.dma_start(out=W2sb[:], in_=W2h[:])
            D2sb = wp.tile([H + 1, T * H], f32)
            nc.sync.dma_start(out=D2sb[:], in_=D2h[:])
            W3sb = wp.tile([H + 1, T * O], f32)
            nc.sync.dma_start(out=W3sb[:], in_=W3h[:])
            nthsb = wp.tile([H, T], f32)
            nc.sync.dma_start(out=nthsb[:], in_=NTh[:])
            cur1 = wp.tile([H, BL], f32)

            # ---- Phase 1: cur1 = w1 @ xT + b1, in two column halves ----
            with (
                tc.tile_pool(name="fc1x", bufs=3) as xp,
                tc.tile_pool(name="fc1ps", bufs=1, space="PSUM") as pp1,
            ):
                for h in range(2):
                    ps = pp1.tile([H, FH], f32, tag="fc1")
                    for k in range(NK):
                        xt = xp.tile([DCH, FH], f32, tag="xt")
                        nc.gpsimd.dma_start(
                            out=xt[:],
                            in_=xT[k * DCH:(k + 1) * DCH, h * FH:(h + 1) * FH],
                        )
                        for s in range(FH // 512):
                            nc.tensor.matmul(
                                ps[:, s * 512:(s + 1) * 512],
                                lhsT=w1sb[:, k * H:(k + 1) * H],
                                rhs=xt[:, s * 512:(s + 1) * 512],
                                start=(k == 0),
                                stop=(k == NK - 1),
                            )
                    nc.scalar.activation(
                        cur1[:, h * FH:(h + 1) * FH], ps[:],
                        AF.Identity, bias=b1sb[:], scale=1.0,
                    )

            # ---- Phase 2: the 25-step scan, per column chunk ----
            with (
                tc.tile_pool(name="spk", bufs=2) as sp,
                tc.tile_pool(name="ps2", bufs=1, space="PSUM") as pp2,
                tc.tile_pool(name="t3ps", bufs=2, space="PSUM") as pp3,
                tc.tile_pool(name="st3", bufs=3) as st3p,
            ):
                for c in range(NCHUNK):
                    c0 = c * BC
                    n1 = pp2.tile([H, BC], f32, tag="n1")
                    n2 = pp2.tile([H, BC], f32, tag="n2")
                    spk1 = sp.tile([H + 1, BC], f32, tag="spk1")
                    spk2s = sp.tile([H + 1, BC], f32, tag="spk2s")
                    nc.sync.dma_start(out=spk1[H:H + 1, :], in_=ONEh[:])
                    nc.sync.dma_start(out=spk2s[H:H + 1, :], in_=ONEh[:])

                    t3 = None
                    for t in range(T):
                        tsl = slice(t * H, (t + 1) * H)
                        j = t % 4
                        if j == 0:
                            t3 = pp3.tile([128, BC], f32, tag="t3")
                        # A: n1 += beta^-t * cur1
                        for s in range(NSL):
                            ssl = slice(s * 512, (s + 1) * 512)
                            nc.tensor.matmul(
                                n1[:, ssl], lhsT=ADsb[:, tsl],
                                rhs=cur1[:, c0 + s * 512:c0 + (s + 1) * 512],
                                start=(t == 0), stop=(t == T - 1),
                            )
                        # s1: spk1 = (n1 > beta^-t)
                        nc.vector.tensor_scalar(
                            spk1[0:H, :], n1[:], float(_binv(t)), None, ALU.is_gt,
                        )
                        # C1: n2 += beta^-t * (w2 @ spk1 + b2)
                        for s in range(NSL):
                            ssl = slice(s * 512, (s + 1) * 512)
                            nc.tensor.matmul(
                                n2[:, ssl], lhsT=W2sb[:, tsl], rhs=spk1[:, ssl],
                                start=(t == 0), stop=(t == T - 1),
                            )
                        # C2: n1 -= beta^-(t+1) * spk1
                        if t < T - 1:
                            for s in range(NSL):
                                ssl = slice(s * 512, (s + 1) * 512)
                                nc.tensor.matmul(
                                    n1[:, ssl], lhsT=D1sb[:, tsl],
                                    rhs=spk1[0:H, ssl],
                                    start=False, stop=False,
                                )
                        # s2: spk2s = sign(n2 - beta^-t)
                        nc.scalar.activation(
                            spk2s[0:H, :], n2[:], AF.Sign,
                            bias=nthsb[:, t:t + 1], scale=1.0,
                        )
                        # E1: n2 -= 0.5*beta^-(t+1) * (spk2s + 1)
                        if t < T - 1:
                            for s in range(NSL):
                                ssl = slice(s * 512, (s + 1) * 512)
                                nc.tensor.matmul(
                                    n2[:, ssl], lhsT=D2sb[:, tsl],
                                    rhs=spk2s[:, ssl],
                                    start=False, stop=False,
                                )
                        # E2: cur3_t into partition slot 32j of t3
                        for s in range(NSL):
                            ssl = slice(s * 512, (s + 1) * 512)
                            nc.tensor.matmul(
                                t3[32 * j:32 * j + O, ssl],
                                lhsT=W3sb[:, t * O:(t + 1) * O],
                                rhs=spk2s[:, ssl],
                                start=True, stop=True,
                                tile_position=(0, 32 * j),
                            )
                        # close a 4-step group: evacuate t3
                        if j == 3 or t == T - 1:
                            g = t // 4
                            stage = st3p.tile([128, BC], f32, tag="stage")
                            nc.scalar.activation(stage[:], t3[:], AF.Copy)
                            for jj in range(j + 1):
                                tt = 4 * g + jj
                                nc.sync.dma_start(
                                    out=cur3[tt, :, c0:c0 + BC],
                                    in_=stage[32 * jj:32 * jj + O, :],
                                )

    # Walrus codegen allows at most one semaphore wait per instruction;
    # split/migrate excess waits the same way Bacc.compile() does.
    import bass_rust as _bass_rust
    _bass_rust.move_matmul_waits_to_ldweights(nc.m)
    _bass_rust.generate_event_semaphores(nc)
    return nc


def _get_program():
    if "nc" not in _CACHE:
        _CACHE["nc"] = _build_program()
    return _CACHE["nc"]


def _host_weights(w1, b1, w2, b2, w3, b3):
    f64 = np.float64
    w1T = np.ascontiguousarray(w1.T)                       # [784, 100]
    w1p = np.empty((DCH, NK * H), np.float32)
    for k in range(NK):
        w1p[:, k * H:(k + 1) * H] = w1T[k * DCH:(k + 1) * DCH]
    b1c = np.ascontiguousarray(b1.reshape(H, 1))

    ADh = np.zeros((H, T * H), np.float32)
    D1h = np.zeros((H, T * H), np.float32)
    W2h = np.zeros((H + 1, T * H), np.float32)
    D2h = np.zeros((H + 1, T * H), np.float32)
    W3h = np.zeros((H + 1, T * O), np.float32)
    idx = np.arange(H)
    for t in range(T):
        bt = f64(BETA) ** (-t)
        btn = f64(BETA) ** (-(t + 1))
        ADh[idx, t * H + idx] = np.float32(bt)
        D1h[idx, t * H + idx] = np.float32(-btn)
        W2h[0:H, t * H:(t + 1) * H] = (bt * w2.astype(f64)).T.astype(np.float32)
        W2h[H, t * H:(t + 1) * H] = (bt * b2.astype(f64)).astype(np.float32)
        D2h[idx, t * H + idx] = np.float32(-0.5 * btn)
        D2h[H, t * H:(t + 1) * H] = np.float32(-0.5 * btn)
        W3h[0:H, t * O:(t + 1) * O] = (0.5 * w3.astype(f64)).T.astype(np.float32)
        W3h[H, t * O:(t + 1) * O] = (
            b3.astype(f64) + 0.5 * w3.astype(f64).sum(axis=1)
        ).astype(np.float32)
    NTh = np.empty((H, T), np.float32)
    for t in range(T):
        NTh[:, t] = -_binv(t)
    ONEh = np.ones((1, BC), np.float32)
    return dict(w1p=w1p, b1c=b1c, ADh=ADh, D1h=D1h, W2h=W2h, D2h=D2h,
                W3h=W3h, NTh=NTh, ONEh=ONEh)


def _host_lif3(cur3_full):
    """cur3_full: [T, B, 2] fp32 -> (spk3_rec, mem3_rec) each [T, B, 2]."""
    f32 = np.float32
    n = cur3_full.shape[1]
    m3 = np.zeros((n, O), f32)
    spk3 = np.zeros((T, n, O), f32)
    mem3 = np.zeros((T, n, O), f32)
    rst = np.zeros((n, O), f32)
    beta = f32(BETA)
    th = f32(TH)
    for t in range(T):
        m3 = beta * m3 + cur3_full[t] - rst * th
        s = ((m3 - th) > 0).astype(f32)
        spk3[t] = s
        mem3[t] = m3
        rst = s
    return spk3, mem3


def kernel(x, w1, b1, w2, b2, w3, b3):
    from concourse.bass_utils import run_bass_kernel_spmd

    x = np.asarray(x, np.float32)
    shared = _host_weights(
        np.asarray(w1, np.float32), np.asarray(b1, np.float32),
        np.asarray(w2, np.float32), np.asarray(b2, np.float32),
        np.asarray(w3, np.float32), np.asarray(b3, np.float32),
    )
    in_maps = []
    for i in range(NCORES):
        xi = np.ascontiguousarray(x[i * BL:(i + 1) * BL].T)  # [784, 8192]
        m = dict(shared)
        m["xT"] = xi
        in_maps.append(m)

    nc = _get_program()
    res = run_bass_kernel_spmd(nc, in_maps, core_ids=list(range(NCORES)))

    cur3_full = np.empty((T, B, O), np.float32)
    for i in range(NCORES):
        ci = res.results[i]["cur3"]          # [T, 2, BL]
        cur3_full[:, i * BL:(i + 1) * BL, :] = np.transpose(ci, (0, 2, 1))

    return _host_lif3(cur3_full)


# revision 16
# speedup vs baseline: 1.2453x; 1.2453x over previous
"""Trainium2 Bass kernel for a 3-layer LIF spiking net (25-step temporal scan).

Strategy (pure data parallel over batch, 8 cores):
  - Host transposes/shards x, precomputes all per-step scaled weight matrices.
  - Device (per core, B_local=8192): fc1 = w1@xT+b1 once; then a 25-step scan
    with membranes kept PSUM-resident in a scaled basis n_t = beta^(-t) * m_t,
    so the per-step decay becomes pure PSUM accumulation:
      n1 += beta^-t * cur1                      (PE, diag lhsT)
      spk1 = (n1 > beta^-t)                     (DVE is_gt, {0,1})
      n2 += beta^-t * (w2 @ spk1 + b2)          (PE)
      n1 -= beta^-(t+1) * spk1                  (PE, diag lhsT)
      spk2s = sign(n2 - beta^-t)                (ACT Sign, {-1,+1})
      n2 -= 0.5*beta^-(t+1) * (spk2s + 1)       (PE)
      cur3_t = 0.5*w3 @ spk2s + (b3 + 0.5*w3@1) (PE, natural units, packed into
                                                 32-aligned PSUM partition slots)
  - cur3 [25, 2, B] goes back to host; the tiny layer-3 LIF scan runs in numpy
    (exact fp32 semantics), producing spk3_rec / mem3_rec.
"""

import numpy as np

BETA = 0.9
T = 25
TH = 1.0
B, D, H, O = 65536, 784, 100, 2
NCORES = 8
BL = B // NCORES          # 8192 per core
BC = 1024                 # scan column chunk
NCHUNK = BL // BC         # 8
NSL = BC // 512           # 512-col sub-matmuls per chunk
DCH = 112                 # fc1 contraction chunk (784 = 7*112)
NK = D // DCH             # 7
FH = BL // 2              # fc1 column half (4096)

_CACHE = {}


def _binv(t):
    return np.float32(np.float64(BETA) ** (-t))


def _build_program(repeat=1):
    import concourse.bass as bass
    import concourse.mybir as mybir
    from concourse.tile import TileContext

    f32 = mybir.dt.float32
    AF = mybir.ActivationFunctionType
    ALU = mybir.AluOpType

    nc = bass.Bass()

    xT = nc.dram_tensor("xT", [D, BL], f32, kind="ExternalInput")
    w1p = nc.dram_tensor("w1p", [DCH, NK * H], f32, kind="ExternalInput")
    b1c = nc.dram_tensor("b1c", [H, 1], f32, kind="ExternalInput")
    ADh = nc.dram_tensor("ADh", [H, T * H], f32, kind="ExternalInput")
    D1h = nc.dram_tensor("D1h", [H, T * H], f32, kind="ExternalInput")
    W2h = nc.dram_tensor("W2h", [H + 1, T * H], f32, kind="ExternalInput")
    D2h = nc.dram_tensor("D2h", [H + 1, T * H], f32, kind="ExternalInput")
    W3h = nc.dram_tensor("W3h", [H + 1, T * O], f32, kind="ExternalInput")
    NTh = nc.dram_tensor("NTh", [H, T], f32, kind="ExternalInput")
    ONEh = nc.dram_tensor("ONEh", [1, BC], f32, kind="ExternalInput")
    cur3 = nc.dram_tensor("cur3", [T, O, BL], f32, kind="ExternalOutput")

    with TileContext(nc) as tc:
        with tc.tile_pool(name="wpool", bufs=1) as wp:
            w1sb = wp.tile([DCH, NK * H], f32)
            nc.sync.dma_start(out=w1sb[:], in_=w1p[:])
            b1sb = wp.tile([H, 1], f32)
            nc.sync.dma_start(out=b1sb[:], in_=b1c[:])
            ADsb = wp.tile([H, T * H], f32)
            nc.sync.dma_start(out=ADsb[:], in_=ADh[:])
            D1sb = wp.tile([H, T * H], f32)
            nc.sync.dma_start(out=D1sb[:], in_=D1h[:])
            W2sb = wp.tile([H + 1, T * H], f32)
            nc.sync.dma_start(out=W2sb[:], in_=W2h[:])
            D2sb = wp.tile([H + 1, T * H], f32)
            nc.sync.dma_start(out=D2sb[:], in_=D2h[:])
            W3sb = wp.tile([H + 1, T * O], f32)
            nc.sync.dma_start(out=W3sb[:], in_=W3h[:])
            nthsb = wp.tile([H, T], f32)
            nc.sync.dma_start(out=nthsb[:], in_=NTh[:])
            cur1 = wp.tile([H, BL], f32)

            # ---- Phase 1: cur1 = w1 @ xT + b1, in two column halves ----
            def _phase1():
              with (
                tc.tile_pool(name="fc1x", bufs=3) as xp,
                tc.tile_pool(name="fc1ps", bufs=1, space="PSUM") as pp1,
              ):
                for h in range(2):
                    ps = pp1.tile([H, FH], f32, tag="fc1")
                    for k in range(NK):
                        xt = xp.tile([DCH, FH], f32, tag="xt")
                        nc.gpsimd.dma_start(
                            out=xt[:],
                            in_=xT[k * DCH:(k + 1) * DCH, h * FH:(h + 1) * FH],
                        )
                        for s in range(FH // 512):
                            nc.tensor.matmul(
                                ps[:, s * 512:(s + 1) * 512],
                                lhsT=w1sb[:, k * H:(k + 1) * H],
                                rhs=xt[:, s * 512:(s + 1) * 512],
                                start=(k == 0),
                                stop=(k == NK - 1),
                            )
                    nc.scalar.activation(
                        cur1[:, h * FH:(h + 1) * FH], ps[:],
                        AF.Identity, bias=b1sb[:], scale=1.0,
                    )

            # ---- Phase 2: the 25-step scan, per column chunk ----
            def _phase2():
              with (
                tc.tile_pool(name="spk", bufs=2) as sp,
                tc.tile_pool(name="ps2", bufs=1, space="PSUM") as pp2,
                tc.tile_pool(name="t3ps", bufs=2, space="PSUM") as pp3,
                tc.tile_pool(name="st3", bufs=3) as st3p,
              ):
                for c in range(NCHUNK):
                    c0 = c * BC
                    n1 = pp2.tile([H, BC], f32, tag="n1")
                    n2 = pp2.tile([H, BC], f32, tag="n2")
                    spk1 = sp.tile([H + 1, BC], f32, tag="spk1")
                    spk2s = sp.tile([H + 1, BC], f32, tag="spk2s")
                    nc.sync.dma_start(out=spk1[H:H + 1, :], in_=ONEh[:])
                    nc.sync.dma_start(out=spk2s[H:H + 1, :], in_=ONEh[:])

                    t3 = None
                    for t in range(T):
                        tsl = slice(t * H, (t + 1) * H)
                        j = t % 4
                        if j == 0:
                            t3 = pp3.tile([128, BC], f32, tag="t3")
                        # A: n1 += beta^-t * cur1
                        for s in range(NSL):
                            ssl = slice(s * 512, (s + 1) * 512)
                            nc.tensor.matmul(
                                n1[:, ssl], lhsT=ADsb[:, tsl],
                                rhs=cur1[:, c0 + s * 512:c0 + (s + 1) * 512],
                                start=(t == 0), stop=(t == T - 1),
                            )
                        # s1: spk1 = (n1 > beta^-t)
                        nc.vector.tensor_scalar(
                            spk1[0:H, :], n1[:], float(_binv(t)), None, ALU.is_gt,
                        )
                        # C1: n2 += beta^-t * (w2 @ spk1 + b2)
                        for s in range(NSL):
                            ssl = slice(s * 512, (s + 1) * 512)
                            nc.tensor.matmul(
                                n2[:, ssl], lhsT=W2sb[:, tsl], rhs=spk1[:, ssl],
                                start=(t == 0), stop=(t == T - 1),
                            )
                        # C2: n1 -= beta^-(t+1) * spk1
                        if t < T - 1:
                            for s in range(NSL):
                                ssl = slice(s * 512, (s + 1) * 512)
                                nc.tensor.matmul(
                                    n1[:, ssl], lhsT=D1sb[:, tsl],
                                    rhs=spk1[0:H, ssl],
                                    start=False, stop=False,
                                )
                        # s2: spk2s = sign(n2 - beta^-t)
                        nc.scalar.activation(
                            spk2s[0:H, :], n2[:], AF.Sign,
                            bias=nthsb[:, t:t + 1], scale=1.0,
                        )
                        # E1: n2 -= 0.5*beta^-(t+1) * (spk2s + 1)
                        if t < T - 1:
                            for s in range(NSL):
                                ssl = slice(s * 512, (s + 1) * 512)
                                nc.tensor.matmul(
                                    n2[:, ssl], lhsT=D2sb[:, tsl],
                                    rhs=spk2s[:, ssl],
                                    start=False, stop=False,
                                )
                        # E2: cur3_t into partition slot 32j of t3
                        for s in range(NSL):
                            ssl = slice(s * 512, (s + 1) * 512)
                            nc.tensor.matmul(
                                t3[32 * j:32 * j + O, ssl],
                                lhsT=W3sb[:, t * O:(t + 1) * O],
                                rhs=spk2s[:, ssl],
                                start=True, stop=True,
                                tile_position=(0, 32 * j),
                            )
                        # close a 4-step group: evacuate t3
                        if j == 3 or t == T - 1:
                            g = t // 4
                            stage = st3p.tile([128, BC], f32, tag="stage")
                            nc.scalar.activation(stage[:], t3[:], AF.Copy)
                            for jj in range(j + 1):
                                tt = 4 * g + jj
                                nc.sync.dma_start(
                                    out=cur3[tt, :, c0:c0 + BC],
                                    in_=stage[32 * jj:32 * jj + O, :],
                                )

            for _rep in range(repeat):
                _phase1()
                _phase2()

    # Walrus codegen allows at most one semaphore wait per instruction;
    # split/migrate excess waits the same way Bacc.compile() does.
    import bass_rust as _bass_rust
    _bass_rust.move_matmul_waits_to_ldweights(nc.m)
    _bass_rust.generate_event_semaphores(nc)
    return nc


def _get_program(repeat=1):
    key = f"nc{repeat}"
    if key not in _CACHE:
        _CACHE[key] = _build_program(repeat)
    return _CACHE[key]


def _host_weights(w1, b1, w2, b2, w3, b3):
    f64 = np.float64
    w1T = np.ascontiguousarray(w1.T)                       # [784, 100]
    w1p = np.empty((DCH, NK * H), np.float32)
    for k in range(NK):
        w1p[:, k * H:(k + 1) * H] = w1T[k * DCH:(k + 1) * DCH]
    b1c = np.ascontiguousarray(b1.reshape(H, 1))

    ADh = np.zeros((H, T * H), np.float32)
    D1h = np.zeros((H, T * H), np.float32)
    W2h = np.zeros((H + 1, T * H), np.float32)
    D2h = np.zeros((H + 1, T * H), np.float32)
    W3h = np.zeros((H + 1, T * O), np.float32)
    idx = np.arange(H)
    for t in range(T):
        bt = f64(BETA) ** (-t)
        btn = f64(BETA) ** (-(t + 1))
        ADh[idx, t * H + idx] = np.float32(bt)
        D1h[idx, t * H + idx] = np.float32(-btn)
        W2h[0:H, t * H:(t + 1) * H] = (bt * w2.astype(f64)).T.astype(np.float32)
        W2h[H, t * H:(t + 1) * H] = (bt * b2.astype(f64)).astype(np.float32)
        D2h[idx, t * H + idx] = np.float32(-0.5 * btn)
        D2h[H, t * H:(t + 1) * H] = np.float32(-0.5 * btn)
        W3h[0:H, t * O:(t + 1) * O] = (0.5 * w3.astype(f64)).T.astype(np.float32)
        W3h[H, t * O:(t + 1) * O] = (
            b3.astype(f64) + 0.5 * w3.astype(f64).sum(axis=1)
        ).astype(np.float32)
    NTh = np.empty((H, T), np.float32)
    for t in range(T):
        NTh[:, t] = -_binv(t)
    ONEh = np.ones((1, BC), np.float32)
    return dict(w1p=w1p, b1c=b1c, ADh=ADh, D1h=D1h, W2h=W2h, D2h=D2h,
                W3h=W3h, NTh=NTh, ONEh=ONEh)


def _host_lif3(cur3_full):
    """cur3_full: [T, B, 2] fp32 -> (spk3_rec, mem3_rec) each [T, B, 2]."""
    f32 = np.float32
    n = cur3_full.shape[1]
    m3 = np.zeros((n, O), f32)
    spk3 = np.zeros((T, n, O), f32)
    mem3 = np.zeros((T, n, O), f32)
    rst = np.zeros((n, O), f32)
    beta = f32(BETA)
    th = f32(TH)
    for t in range(T):
        m3 = beta * m3 + cur3_full[t] - rst * th
        s = ((m3 - th) > 0).astype(f32)
        spk3[t] = s
        mem3[t] = m3
        rst = s
    return spk3, mem3


def kernel(x, w1, b1, w2, b2, w3, b3):
    from concourse.bass_utils import run_bass_kernel_spmd

    x = np.asarray(x, np.float32)
    shared = _host_weights(
        np.asarray(w1, np.float32), np.asarray(b1, np.float32),
        np.asarray(w2, np.float32), np.asarray(b2, np.float32),
        np.asarray(w3, np.float32), np.asarray(b3, np.float32),
    )
    in_maps = []
    for i in range(NCORES):
        xi = np.ascontiguousarray(x[i * BL:(i + 1) * BL].T)  # [784, 8192]
        m = dict(shared)
        m["xT"] = xi
        in_maps.append(m)

    nc = _get_program()
    res = run_bass_kernel_spmd(nc, in_maps, core_ids=list(range(NCORES)))

    cur3_full = np.empty((T, B, O), np.float32)
    for i in range(NCORES):
        ci = res.results[i]["cur3"]          # [T, 2, BL]
        cur3_full[:, i * BL:(i + 1) * BL, :] = np.transpose(ci, (0, 2, 1))

    return _host_lif3(cur3_full)


# revision 24
# speedup vs baseline: 6.7787x; 5.4435x over previous
"""Trainium2 Bass kernel for a 3-layer LIF spiking net (25-step temporal scan).

Strategy (pure data parallel over batch, 8 cores):
  - Host transposes/shards x, precomputes all per-step scaled weight matrices.
  - Device (per core, B_local=8192): fc1 = w1@xT+b1 once; then a 25-step scan
    with membranes kept PSUM-resident in a scaled basis n_t = beta^(-t) * m_t,
    so the per-step decay becomes pure PSUM accumulation:
      n1 += beta^-t * cur1                      (PE, diag lhsT)
      spk1 = (n1 > beta^-t)                     (DVE is_gt, {0,1})
      n2 += beta^-t * (w2 @ spk1 + b2)          (PE)
      n1 -= beta^-(t+1) * spk1                  (PE, diag lhsT)
      spk2s = sign(n2 - beta^-t)                (ACT Sign, {-1,+1})
      n2 -= 0.5*beta^-(t+1) * (spk2s + 1)       (PE)
      cur3_t = 0.5*w3 @ spk2s + (b3 + 0.5*w3@1) (PE, natural units, packed into
                                                 32-aligned PSUM partition slots)
  - cur3 [25, 2, B] goes back to host; the tiny layer-3 LIF scan runs in numpy
    (exact fp32 semantics), producing spk3_rec / mem3_rec.
"""

import numpy as np

BETA = 0.9
T = 25
TH = 1.0
B, D, H, O = 65536, 784, 100, 2
NCORES = 8
BL = B // NCORES          # 8192 per core
BC = 1024                 # scan column chunk
NCHUNK = BL // BC         # 8
NSL = BC // 512           # 512-col sub-matmuls per chunk
DCH = 112                 # fc1 contraction chunk (784 = 7*112)
NK = D // DCH             # 7
FH = BL // 2              # fc1 column half (4096)

_CACHE = {}


def _binv(t):
    return np.float32(np.float64(BETA) ** (-t))


def _build_program(repeat=1):
    import concourse.bass as bass
    import concourse.mybir as mybir
    from concourse.tile import TileContext

    f32 = mybir.dt.float32
    AF = mybir.ActivationFunctionType
    ALU = mybir.AluOpType

    nc = bass.Bass()

    bf16 = mybir.dt.bfloat16
    xT = nc.dram_tensor("xT", [D, BL], f32, kind="ExternalInput")
    w1p = nc.dram_tensor("w1p", [DCH, NK * H], f32, kind="ExternalInput")
    b1c = nc.dram_tensor("b1c", [H, 1], f32, kind="ExternalInput")
    ADh = nc.dram_tensor("ADh", [H, T * H], f32, kind="ExternalInput")
    # bf16 hi/lo pairs for the scan matmul weights
    scan_w = {}
    for nm, rows in [("D1", H), ("W2", H + 1), ("D2", H + 1)]:
        for p in ("h", "l"):
            scan_w[nm + p] = nc.dram_tensor(
                nm + p, [rows, T * H], bf16, kind="ExternalInput")
    for p in ("h", "l"):
        scan_w["W3" + p] = nc.dram_tensor(
            "W3" + p, [H + 1, T * O], bf16, kind="ExternalInput")
    NTh = nc.dram_tensor("NTh", [H, T], f32, kind="ExternalInput")
    ONEh = nc.dram_tensor("ONEh", [1, BC], bf16, kind="ExternalInput")
    cur3 = nc.dram_tensor("cur3", [T, O, BL], f32, kind="ExternalOutput")

    with TileContext(nc) as tc:
        with tc.tile_pool(name="wpool", bufs=1) as wp:
            w1sb = wp.tile([DCH, NK * H], f32)
            nc.sync.dma_start(out=w1sb[:], in_=w1p[:])
            b1sb = wp.tile([H, 1], f32)
            nc.sync.dma_start(out=b1sb[:], in_=b1c[:])
            ADsb = wp.tile([H, T * H], f32)
            nc.sync.dma_start(out=ADsb[:], in_=ADh[:])
            wsb = {}
            for nm, dram in scan_w.items():
                wsb[nm] = wp.tile(list(dram.shape), bf16, name="wsb_" + nm)
                nc.sync.dma_start(out=wsb[nm][:], in_=dram[:])
            nthsb = wp.tile([H, T], f32)
            nc.sync.dma_start(out=nthsb[:], in_=NTh[:])
            cur1 = wp.tile([H, BL], f32)

            # ---- Phase 1: cur1 = w1 @ xT + b1, in two column halves ----
            def _phase1():
              with (
                tc.tile_pool(name="fc1x", bufs=3) as xp,
                tc.tile_pool(name="fc1ps", bufs=1, space="PSUM") as pp1,
              ):
                for h in range(2):
                    ps = pp1.tile([H, FH], f32, tag="fc1")
                    for k in range(NK):
                        xt = xp.tile([DCH, FH], f32, tag="xt")
                        nc.gpsimd.dma_start(
                            out=xt[:],
                            in_=xT[k * DCH:(k + 1) * DCH, h * FH:(h + 1) * FH],
                        )
                        for s in range(FH // 512):
                            nc.tensor.matmul(
                                ps[:, s * 512:(s + 1) * 512],
                                lhsT=w1sb[:, k * H:(k + 1) * H],
                                rhs=xt[:, s * 512:(s + 1) * 512],
                                start=(k == 0),
                                stop=(k == NK - 1),
                            )
                    nc.scalar.activation(
                        cur1[:, h * FH:(h + 1) * FH], ps[:],
                        AF.Identity, bias=b1sb[:], scale=1.0,
                    )

            # ---- Phase 2: the 25-step scan, per column chunk ----
            def _phase2():
              with (
                tc.tile_pool(name="spk", bufs=2) as sp,
                tc.tile_pool(name="ps2", bufs=1, space="PSUM") as pp2,
                tc.tile_pool(name="t3ps", bufs=2, space="PSUM") as pp3,
                tc.tile_pool(name="st3", bufs=3) as st3p,
              ):
                for c in range(NCHUNK):
                    c0 = c * BC
                    n1 = pp2.tile([H, BC], f32, tag="n1")
                    n2 = pp2.tile([H, BC], f32, tag="n2")
                    spk1 = sp.tile([H + 1, BC], bf16, tag="spk1")
                    spk2s = sp.tile([H + 1, BC], bf16, tag="spk2s")
                    nc.sync.dma_start(out=spk1[H:H + 1, :], in_=ONEh[:])
                    nc.sync.dma_start(out=spk2s[H:H + 1, :], in_=ONEh[:])

                    t3 = None
                    for t in range(T):
                        tsl = slice(t * H, (t + 1) * H)
                        j = t % 4
                        if j == 0:
                            t3 = pp3.tile([128, BC], f32, tag="t3")
                        # A: n1 += beta^-t * cur1
                        for s in range(NSL):
                            ssl = slice(s * 512, (s + 1) * 512)
                            nc.tensor.matmul(
                                n1[:, ssl], lhsT=ADsb[:, tsl],
                                rhs=cur1[:, c0 + s * 512:c0 + (s + 1) * 512],
                                start=(t == 0), stop=(t == T - 1),
                            )
                        # s1: spk1 = (n1 > beta^-t)
                        nc.vector.tensor_scalar(
                            spk1[0:H, :], n1[:], float(_binv(t)), None, ALU.is_gt,
                        )
                        # C1: n2 += beta^-t * (w2 @ spk1 + b2)  [bf16 hi+lo]
                        for s in range(NSL):
                            ssl = slice(s * 512, (s + 1) * 512)
                            nc.tensor.matmul(
                                n2[:, ssl], lhsT=wsb["W2h"][:, tsl],
                                rhs=spk1[:, ssl],
                                start=(t == 0), stop=False,
                            )
                            nc.tensor.matmul(
                                n2[:, ssl], lhsT=wsb["W2l"][:, tsl],
                                rhs=spk1[:, ssl],
                                start=False, stop=(t == T - 1),
                            )
                        # C2: n1 -= beta^-(t+1) * spk1  [bf16 hi+lo]
                        if t < T - 1:
                            for s in range(NSL):
                                ssl = slice(s * 512, (s + 1) * 512)
                                for p in ("D1h", "D1l"):
                                    nc.tensor.matmul(
                                        n1[:, ssl], lhsT=wsb[p][:, tsl],
                                        rhs=spk1[0:H, ssl],
                                        start=False, stop=False,
                                    )
                        # s2: spk2s = sign(n2 - beta^-t)
                        nc.scalar.activation(
                            spk2s[0:H, :], n2[:], AF.Sign,
                            bias=nthsb[:, t:t + 1], scale=1.0,
                        )
                        # E1: n2 -= 0.5*beta^-(t+1) * (spk2s + 1)  [bf16 hi+lo]
                        if t < T - 1:
                            for s in range(NSL):
                                ssl = slice(s * 512, (s + 1) * 512)
                                for p in ("D2h", "D2l"):
                                    nc.tensor.matmul(
                                        n2[:, ssl], lhsT=wsb[p][:, tsl],
                                        rhs=spk2s[:, ssl],
                                        start=False, stop=False,
                                    )
                        # E2: cur3_t into partition slot 32j of t3  [bf16 hi+lo]
                        for s in range(NSL):
                            ssl = slice(s * 512, (s + 1) * 512)
                            nc.tensor.matmul(
                                t3[32 * j:32 * j + O, ssl],
                                lhsT=wsb["W3h"][:, t * O:(t + 1) * O],
                                rhs=spk2s[:, ssl],
                                start=True, stop=False,
                                tile_position=(0, 32 * j),
                            )
                            nc.tensor.matmul(
                                t3[32 * j:32 * j + O, ssl],
                                lhsT=wsb["W3l"][:, t * O:(t + 1) * O],
                                rhs=spk2s[:, ssl],
                                start=False, stop=True,
                                tile_position=(0, 32 * j),
                            )
                        # close a 4-step group: evacuate t3
                        if j == 3 or t == T - 1:
                            g = t // 4
                            stage = st3p.tile([128, BC], f32, tag="stage")
                            nc.scalar.activation(stage[:], t3[:], AF.Copy)
                            for jj in range(j + 1):
                                tt = 4 * g + jj
                                nc.sync.dma_start(
                                    out=cur3[tt, :, c0:c0 + BC],
                                    in_=stage[32 * jj:32 * jj + O, :],
                                )

            for _rep in range(repeat):
                _phase1()
                _phase2()

    # Walrus codegen allows at most one semaphore wait per instruction;
    # split/migrate excess waits the same way Bacc.compile() does.
    import bass_rust as _bass_rust
    _bass_rust.move_matmul_waits_to_ldweights(nc.m)
    _bass_rust.generate_event_semaphores(nc)
    return nc


def _get_program(repeat=1):
    key = f"nc{repeat}"
    if key not in _CACHE:
        _CACHE[key] = _build_program(repeat)
    return _CACHE[key]


def _split_bf16(M):
    import ml_dtypes
    bf = ml_dtypes.bfloat16
    hi = M.astype(bf)
    lo = (M - hi.astype(np.float32)).astype(bf)
    return hi, lo


def _host_weights(w1, b1, w2, b2, w3, b3):
    f64 = np.float64
    w1T = np.ascontiguousarray(w1.T)                       # [784, 100]
    w1p = np.empty((DCH, NK * H), np.float32)
    for k in range(NK):
        w1p[:, k * H:(k + 1) * H] = w1T[k * DCH:(k + 1) * DCH]
    b1c = np.ascontiguousarray(b1.reshape(H, 1))

    ADh = np.zeros((H, T * H), np.float32)
    D1f = np.zeros((H, T * H), np.float32)
    W2f = np.zeros((H + 1, T * H), np.float32)
    D2f = np.zeros((H + 1, T * H), np.float32)
    W3f = np.zeros((H + 1, T * O), np.float32)
    idx = np.arange(H)
    for t in range(T):
        bt = f64(BETA) ** (-t)
        btn = f64(BETA) ** (-(t + 1))
        ADh[idx, t * H + idx] = np.float32(bt)
        D1f[idx, t * H + idx] = np.float32(-btn)
        W2f[0:H, t * H:(t + 1) * H] = (bt * w2.astype(f64)).T.astype(np.float32)
        W2f[H, t * H:(t + 1) * H] = (bt * b2.astype(f64)).astype(np.float32)
        D2f[idx, t * H + idx] = np.float32(-0.5 * btn)
        D2f[H, t * H:(t + 1) * H] = np.float32(-0.5 * btn)
        W3f[0:H, t * O:(t + 1) * O] = (0.5 * w3.astype(f64)).T.astype(np.float32)
        W3f[H, t * O:(t + 1) * O] = (
            b3.astype(f64) + 0.5 * w3.astype(f64).sum(axis=1)
        ).astype(np.float32)
    out = dict(w1p=w1p, b1c=b1c, ADh=ADh)
    for nm, M in [("D1", D1f), ("W2", W2f), ("D2", D2f), ("W3", W3f)]:
        hi, lo = _split_bf16(M)
        out[nm + "h"] = hi
        out[nm + "l"] = lo
    NTh = np.empty((H, T), np.float32)
    for t in range(T):
        NTh[:, t] = -_binv(t)
    import ml_dtypes
    out["NTh"] = NTh
    out["ONEh"] = np.ones((1, BC), ml_dtypes.bfloat16)
    return out


def _host_lif3(cur3_full):
    """cur3_full: [T, B, 2] fp32 -> (spk3_rec, mem3_rec) each [T, B, 2]."""
    f32 = np.float32
    n = cur3_full.shape[1]
    m3 = np.zeros((n, O), f32)
    spk3 = np.zeros((T, n, O), f32)
    mem3 = np.zeros((T, n, O), f32)
    rst = np.zeros((n, O), f32)
    beta = f32(BETA)
    th = f32(TH)
    for t in range(T):
        m3 = beta * m3 + cur3_full[t] - rst * th
        s = ((m3 - th) > 0).astype(f32)
        spk3[t] = s
        mem3[t] = m3
        rst = s
    return spk3, mem3


def kernel(x, w1, b1, w2, b2, w3, b3):
    from concourse.bass_utils import run_bass_kernel_spmd

    x = np.asarray(x, np.float32)
    shared = _host_weights(
        np.asarray(w1, np.float32), np.asarray(b1, np.float32),
        np.asarray(w2, np.float32), np.asarray(b2, np.float32),
        np.asarray(w3, np.float32), np.asarray(b3, np.float32),
    )
    in_maps = []
    for i in range(NCORES):
        xi = np.ascontiguousarray(x[i * BL:(i + 1) * BL].T)  # [784, 8192]
        m = dict(shared)
        m["xT"] = xi
        in_maps.append(m)

    nc = _get_program()
    res = run_bass_kernel_spmd(nc, in_maps, core_ids=list(range(NCORES)))

    cur3_full = np.empty((T, B, O), np.float32)
    for i in range(NCORES):
        ci = res.results[i]["cur3"]          # [T, 2, BL]
        cur3_full[:, i * BL:(i + 1) * BL, :] = np.transpose(ci, (0, 2, 1))

    return _host_lif3(cur3_full)


# revision 25
# speedup vs baseline: 45.1118x; 6.6549x over previous
"""Trainium2 Bass kernel for a 3-layer LIF spiking net (25-step temporal scan).

Strategy (pure data parallel over batch, 8 cores):
  - Host transposes/shards x and precomputes per-step scaled weights.
  - Device per core (B_local=8192): fc1 = w1@xT+b1 once (fp32). The 25-step
    scan keeps layer state PSUM-resident in a scaled basis n_t = beta^-t*m_t:
      layer1 is reduced to a spike-sum S_t = sum_tau beta^-(tau+1)*spk1_tau
      (PSUM) plus a closed-form drive threshold R_t = c_t*cur1 - beta^-t
      (DVE), so no per-step membrane update is needed:
        R_t   = c_t*cur1 - beta^-t          (DVE tensor_scalar, fp32)
        spk1  = (R_t > S)                   (DVE is_gt -> bf16 {0,1})
        S    += beta^-(t+1)*spk1            (PE, fp16 hi/lo diag)
        n2   += beta^-t*w2 @ spk1           (PE, bf16 hi/lo)
        spk2s = sign(n2 - theta2_t[h])      (ACT Sign -> bf16 {+-1};
                                             theta2 folds b2 and the sym
                                             constant, host-precomputed)
        n2   -= 0.5*beta^-(t+1)*spk2s       (PE, fp16 hi/lo diag)
        cur3 += 0.5*w3 @ spk2s              (PE, bf16 hi/lo, packed into
                                             32-aligned PSUM partition slots)
  - cur3 [25, 2, B] returns to host; host adds the constant (b3 + 0.5*w3@1)
    and runs the tiny layer-3 LIF scan in exact fp32 numpy.
"""

import numpy as np

BETA = 0.9
T = 25
TH = 1.0
B, D, H, O = 65536, 784, 100, 2
NCORES = 8
BL = B // NCORES          # 8192 per core
BC = 1024                 # scan column chunk
NCHUNK = BL // BC         # 8
NSL = BC // 512           # 512-col sub-matmuls per chunk
DCH = 112                 # fc1 contraction chunk (784 = 7*112)
NK = D // DCH             # 7
FH = BL // 2              # fc1 column half (4096)

_CACHE = {}


def _f64(v):
    return np.float64(v)


def _binv(t):
    return np.float32(_f64(BETA) ** (-t))


def _csum(t):
    return float(sum(_f64(BETA) ** (-u) for u in range(t + 1)))


def _build_program(repeat=1):
    import concourse.bass as bass
    import concourse.mybir as mybir
    from concourse.tile import TileContext

    f32 = mybir.dt.float32
    bf16 = mybir.dt.bfloat16
    f16 = mybir.dt.float16
    AF = mybir.ActivationFunctionType
    ALU = mybir.AluOpType

    nc = bass.Bass()

    xT = nc.dram_tensor("xT", [D, BL], f32, kind="ExternalInput")
    w1p = nc.dram_tensor("w1p", [DCH, NK * H], f32, kind="ExternalInput")
    b1c = nc.dram_tensor("b1c", [H, 1], f32, kind="ExternalInput")
    W2a = nc.dram_tensor("W2a", [H, T * H], bf16, kind="ExternalInput")
    W2b = nc.dram_tensor("W2b", [H, T * H], bf16, kind="ExternalInput")
    D1a = nc.dram_tensor("D1a", [H, T * H], f16, kind="ExternalInput")
    D1b = nc.dram_tensor("D1b", [H, T * H], f16, kind="ExternalInput")
    D2a = nc.dram_tensor("D2a", [H, T * H], f16, kind="ExternalInput")
    D2b = nc.dram_tensor("D2b", [H, T * H], f16, kind="ExternalInput")
    W3a = nc.dram_tensor("W3a", [H, T * O], bf16, kind="ExternalInput")
    W3b = nc.dram_tensor("W3b", [H, T * O], bf16, kind="ExternalInput")
    TH2 = nc.dram_tensor("TH2", [H, T], f32, kind="ExternalInput")
    cur3 = nc.dram_tensor("cur3", [T, O, BL], f32, kind="ExternalOutput")

    with TileContext(nc) as tc:
        with tc.tile_pool(name="wpool", bufs=1) as wp:
            w1sb = wp.tile([DCH, NK * H], f32)
            nc.sync.dma_start(out=w1sb[:], in_=w1p[:])
            b1sb = wp.tile([H, 1], f32)
            nc.sync.dma_start(out=b1sb[:], in_=b1c[:])
            wsb = {}
            for nm, dram, dt in [
                ("W2a", W2a, bf16), ("W2b", W2b, bf16),
                ("D1a", D1a, f16), ("D1b", D1b, f16),
                ("D2a", D2a, f16), ("D2b", D2b, f16),
                ("W3a", W3a, bf16), ("W3b", W3b, bf16),
            ]:
                wsb[nm] = wp.tile(list(dram.shape), dt, name="wsb_" + nm)
                nc.sync.dma_start(out=wsb[nm][:], in_=dram[:])
            th2sb = wp.tile([H, T], f32)
            nc.sync.dma_start(out=th2sb[:], in_=TH2[:])
            cur1 = wp.tile([H, BL], f32)

            # ---- Phase 1: cur1 = w1 @ xT + b1, fp32, two column halves ----
            def _phase1():
              with (
                tc.tile_pool(name="fc1x", bufs=3) as xp,
                tc.tile_pool(name="fc1ps", bufs=1, space="PSUM") as pp1,
              ):
                for h in range(2):
                    ps = pp1.tile([H, FH], f32, tag="fc1")
                    for k in range(NK):
                        xt = xp.tile([DCH, FH], f32, tag="xt")
                        nc.gpsimd.dma_start(
                            out=xt[:],
                            in_=xT[k * DCH:(k + 1) * DCH, h * FH:(h + 1) * FH],
                        )
                        for s in range(FH // 512):
                            nc.tensor.matmul(
                                ps[:, s * 512:(s + 1) * 512],
                                lhsT=w1sb[:, k * H:(k + 1) * H],
                                rhs=xt[:, s * 512:(s + 1) * 512],
                                start=(k == 0),
                                stop=(k == NK - 1),
                            )
                    nc.scalar.activation(
                        cur1[:, h * FH:(h + 1) * FH], ps[:],
                        AF.Identity, bias=b1sb[:], scale=1.0,
                    )

            # ---- Phase 2: the 25-step scan, per column chunk ----
            def _phase2():
              with (
                tc.tile_pool(name="spk", bufs=2) as sp,
                tc.tile_pool(name="rp", bufs=3) as rp,
                tc.tile_pool(name="ps2", bufs=1, space="PSUM") as pp2,
                tc.tile_pool(name="t3ps", bufs=2, space="PSUM") as pp3,
                tc.tile_pool(name="st3", bufs=3) as st3p,
              ):
                for c in range(NCHUNK):
                    c0 = c * BC
                    S1 = pp2.tile([H, BC], f32, tag="S1")
                    n2 = pp2.tile([H, BC], f32, tag="n2")
                    spk1 = sp.tile([H, BC], bf16, tag="spk1")
                    spk2s = sp.tile([H, BC], bf16, tag="spk2s")

                    t3 = None
                    for t in range(T):
                        tsl = slice(t * H, (t + 1) * H)
                        j = t % 4
                        if j == 0:
                            t3 = pp3.tile([128, BC], f32, tag="t3")
                        # R_t = c_t*cur1 - beta^-t  (DVE, fp32)
                        rt = rp.tile([H, BC], f32, tag="rt")
                        nc.vector.tensor_scalar(
                            rt[:], cur1[:, c0:c0 + BC],
                            _csum(t), -float(_binv(t)), ALU.mult, ALU.add,
                        )
                        # spk1 = (R_t > S)  {0,1} bf16
                        if t == 0:
                            nc.vector.tensor_scalar(
                                spk1[:], rt[:], 0.0, None, ALU.is_gt,
                            )
                        else:
                            nc.vector.tensor_tensor(
                                spk1[:], rt[:], S1[:], ALU.is_gt,
                            )
                        # C2: S += beta^-(t+1)*spk1  [fp16 hi/lo diag]
                        if t < T - 1:
                            for s in range(NSL):
                                ssl = slice(s * 512, (s + 1) * 512)
                                nc.tensor.matmul(
                                    S1[:, ssl], lhsT=wsb["D1a"][:, tsl],
                                    rhs=spk1[:, ssl],
                                    start=(t == 0), stop=False,
                                )
                                nc.tensor.matmul(
                                    S1[:, ssl], lhsT=wsb["D1b"][:, tsl],
                                    rhs=spk1[:, ssl],
                                    start=False, stop=(t == T - 2),
                                )
                        # C1: n2 += beta^-t * w2 @ spk1  [bf16 hi/lo]
                        for s in range(NSL):
                            ssl = slice(s * 512, (s + 1) * 512)
                            nc.tensor.matmul(
                                n2[:, ssl], lhsT=wsb["W2a"][:, tsl],
                                rhs=spk1[:, ssl],
                                start=(t == 0), stop=False,
                            )
                            nc.tensor.matmul(
                                n2[:, ssl], lhsT=wsb["W2b"][:, tsl],
                                rhs=spk1[:, ssl],
                                start=False, stop=(t == T - 1),
                            )
                        # s2: spk2s = sign(n2 - theta2_t)  {-1,+1} bf16
                        nc.scalar.activation(
                            spk2s[:], n2[:], AF.Sign,
                            bias=th2sb[:, t:t + 1], scale=1.0,
                        )
                        # E1: n2 -= 0.5*beta^-(t+1)*spk2s  [fp16 hi/lo diag]
                        if t < T - 1:
                            for s in range(NSL):
                                ssl = slice(s * 512, (s + 1) * 512)
                                for p in ("D2a", "D2b"):
                                    nc.tensor.matmul(
                                        n2[:, ssl], lhsT=wsb[p][:, tsl],
                                        rhs=spk2s[:, ssl],
                                        start=False, stop=False,
                                    )
                        # E2: cur3_t (dev part) into partition slot 32j of t3
                        for s in range(NSL):
                            ssl = slice(s * 512, (s + 1) * 512)
                            nc.tensor.matmul(
                                t3[32 * j:32 * j + O, ssl],
                                lhsT=wsb["W3a"][:, t * O:(t + 1) * O],
                                rhs=spk2s[:, ssl],
                                start=True, stop=False,
                                tile_position=(0, 32 * j),
                            )
                            nc.tensor.matmul(
                                t3[32 * j:32 * j + O, ssl],
                                lhsT=wsb["W3b"][:, t * O:(t + 1) * O],
                                rhs=spk2s[:, ssl],
                                start=False, stop=True,
                                tile_position=(0, 32 * j),
                            )
                        # close a 4-step group: evacuate t3
                        if j == 3 or t == T - 1:
                            g = t // 4
                            stage = st3p.tile([128, BC], f32, tag="stage")
                            nc.scalar.activation(stage[:], t3[:], AF.Copy)
                            for jj in range(j + 1):
                                tt = 4 * g + jj
                                nc.sync.dma_start(
                                    out=cur3[tt, :, c0:c0 + BC],
                                    in_=stage[32 * jj:32 * jj + O, :],
                                )

            for _rep in range(repeat):
                _phase1()
                _phase2()

    # Walrus codegen allows at most one semaphore wait per instruction;
    # split/migrate excess waits the same way Bacc.compile() does.
    import bass_rust as _bass_rust
    _bass_rust.move_matmul_waits_to_ldweights(nc.m)
    _bass_rust.generate_event_semaphores(nc)
    return nc


def _get_program(repeat=1):
    key = f"nc{repeat}"
    if key not in _CACHE:
        _CACHE[key] = _build_program(repeat)
    return _CACHE[key]


def _split2(M, dt):
    hi = M.astype(dt)
    lo = (M - hi.astype(np.float32)).astype(dt)
    return hi, lo


def _host_weights(w1, b1, w2, b2, w3, b3):
    import ml_dtypes
    bf = ml_dtypes.bfloat16
    f16 = np.float16
    f64 = np.float64

    w1T = np.ascontiguousarray(w1.T)                       # [784, 100]
    w1p = np.empty((DCH, NK * H), np.float32)
    for k in range(NK):
        w1p[:, k * H:(k + 1) * H] = w1T[k * DCH:(k + 1) * DCH]
    b1c = np.ascontiguousarray(b1.reshape(H, 1))

    W2f = np.zeros((H, T * H), np.float32)
    D1f = np.zeros((H, T * H), np.float32)
    D2f = np.zeros((H, T * H), np.float32)
    W3f = np.zeros((H, T * O), np.float32)
    TH2 = np.empty((H, T), np.float32)
    idx = np.arange(H)
    for t in range(T):
        bt = f64(BETA) ** (-t)
        btn = f64(BETA) ** (-(t + 1))
        ct = _csum(t)
        W2f[:, t * H:(t + 1) * H] = (bt * w2.astype(f64)).T.astype(np.float32)
        D1f[idx, t * H + idx] = np.float32(btn)
        D2f[idx, t * H + idx] = np.float32(-0.5 * btn)
        W3f[:, t * O:(t + 1) * O] = (0.5 * w3.astype(f64)).T.astype(np.float32)
        # ACT Sign bias: spk2 test is  n2_dev > theta2 = beta^-t - c_t*b2
        #   + 0.5*(c_t - 1); bias = -theta2
        TH2[:, t] = (-(bt - ct * b2.astype(f64) + 0.5 * (ct - 1))).astype(
            np.float32)
    out = dict(w1p=w1p, b1c=b1c, TH2=TH2)
    out["W2a"], out["W2b"] = _split2(W2f, bf)
    out["D1a"], out["D1b"] = _split2(D1f, f16)
    out["D2a"], out["D2b"] = _split2(D2f, f16)
    out["W3a"], out["W3b"] = _split2(W3f, bf)
    return out


def _host_lif3(cur3_dev, w3, b3):
    """cur3_dev: [T, B, 2] fp32 device part -> (spk3_rec, mem3_rec)."""
    f32 = np.float32
    c3 = (b3.astype(np.float64)
          + 0.5 * w3.astype(np.float64).sum(axis=1)).astype(f32)
    n = cur3_dev.shape[1]
    m3 = np.zeros((n, O), f32)
    spk3 = np.zeros((T, n, O), f32)
    mem3 = np.zeros((T, n, O), f32)
    rst = np.zeros((n, O), f32)
    beta = f32(BETA)
    th = f32(TH)
    for t in range(T):
        m3 = beta * m3 + (cur3_dev[t] + c3[None, :]) - rst * th
        s = ((m3 - th) > 0).astype(f32)
        spk3[t] = s
        mem3[t] = m3
        rst = s
    return spk3, mem3


def kernel(x, w1, b1, w2, b2, w3, b3):
    from concourse.bass_utils import run_bass_kernel_spmd

    x = np.asarray(x, np.float32)
    w3 = np.asarray(w3, np.float32)
    b3 = np.asarray(b3, np.float32)
    shared = _host_weights(
        np.asarray(w1, np.float32), np.asarray(b1, np.float32),
        np.asarray(w2, np.float32), np.asarray(b2, np.float32),
        w3, b3,
    )
    in_maps = []
    for i in range(NCORES):
        xi = np.ascontiguousarray(x[i * BL:(i + 1) * BL].T)  # [784, 8192]
        m = dict(shared)
        m["xT"] = xi
        in_maps.append(m)

    nc = _get_program()
    res = run_bass_kernel_spmd(nc, in_maps, core_ids=list(range(NCORES)))

    cur3_dev = np.empty((T, B, O), np.float32)
    for i in range(NCORES):
        ci = res.results[i]["cur3"]          # [T, 2, BL]
        cur3_dev[:, i * BL:(i + 1) * BL, :] = np.transpose(ci, (0, 2, 1))

    return _host_lif3(cur3_dev, w3, b3)


# revision 28
# speedup vs baseline: 60.2588x; 1.3358x over previous
"""Trainium2 Bass kernel for a 3-layer LIF spiking net (25-step temporal scan).

Strategy (pure data parallel over batch, 8 cores):
  - Host transposes/shards x and precomputes per-step scaled weights.
  - Device per core (B_local=8192): fc1 = w1@xT+b1 once (fp32). The 25-step
    scan keeps layer state PSUM-resident in a scaled basis n_t = beta^-t*m_t:
      layer1 is reduced to a spike-sum S_t = sum_tau beta^-(tau+1)*spk1_tau
      (PSUM) plus a closed-form drive threshold R_t = c_t*cur1 - beta^-t
      (DVE), so no per-step membrane update is needed:
        R_t   = c_t*cur1 - beta^-t          (DVE tensor_scalar, fp32)
        spk1  = (R_t > S)                   (DVE is_gt -> bf16 {0,1})
        S    += beta^-(t+1)*spk1            (PE, fp16 hi/lo diag)
        n2   += beta^-t*w2 @ spk1           (PE, bf16 hi/lo)
        spk2s = sign(n2 - theta2_t[h])      (ACT Sign -> bf16 {+-1};
                                             theta2 folds b2 and the sym
                                             constant, host-precomputed)
        n2   -= 0.5*beta^-(t+1)*spk2s       (PE, fp16 hi/lo diag)
        cur3 += 0.5*w3 @ spk2s              (PE, bf16 hi/lo, packed into
                                             32-aligned PSUM partition slots)
  - cur3 [25, 2, B] returns to host; host adds the constant (b3 + 0.5*w3@1)
    and runs the tiny layer-3 LIF scan in exact fp32 numpy.
"""

import numpy as np

BETA = 0.9
T = 25
TH = 1.0
B, D, H, O = 65536, 784, 100, 2
NCORES = 8
BL = B // NCORES          # 8192 per core
BC = 1024                 # scan column chunk
NCHUNK = BL // BC         # 8
NSL = BC // 512           # 512-col sub-matmuls per chunk
DCH = 112                 # fc1 contraction chunk (784 = 7*112)
NK = D // DCH             # 7
FH = BL // 2              # fc1 column half (4096)

_CACHE = {}


def _f64(v):
    return np.float64(v)


def _binv(t):
    return np.float32(_f64(BETA) ** (-t))


def _csum(t):
    return float(sum(_f64(BETA) ** (-u) for u in range(t + 1)))


def _build_program(repeat=1, bc=BC, psum_bufs=1, do_fc1=True, do_scan=True,
                   spk_bufs=2):
    import concourse.bass as bass
    import concourse.mybir as mybir
    from concourse.tile import TileContext

    f32 = mybir.dt.float32
    bf16 = mybir.dt.bfloat16
    f16 = mybir.dt.float16
    AF = mybir.ActivationFunctionType
    ALU = mybir.AluOpType

    nchunk = BL // bc
    nsl = bc // 512

    nc = bass.Bass()

    xT = nc.dram_tensor("xT", [D, BL], f32, kind="ExternalInput")
    w1p = nc.dram_tensor("w1p", [DCH, NK * H], f32, kind="ExternalInput")
    b1c = nc.dram_tensor("b1c", [H, 1], f32, kind="ExternalInput")
    W2a = nc.dram_tensor("W2a", [H, T * H], bf16, kind="ExternalInput")
    W2b = nc.dram_tensor("W2b", [H, T * H], bf16, kind="ExternalInput")
    D1a = nc.dram_tensor("D1a", [H, T * H], f16, kind="ExternalInput")
    D1b = nc.dram_tensor("D1b", [H, T * H], f16, kind="ExternalInput")
    D2a = nc.dram_tensor("D2a", [H, T * H], f16, kind="ExternalInput")
    D2b = nc.dram_tensor("D2b", [H, T * H], f16, kind="ExternalInput")
    W3a = nc.dram_tensor("W3a", [H, T * O], bf16, kind="ExternalInput")
    W3b = nc.dram_tensor("W3b", [H, T * O], bf16, kind="ExternalInput")
    TH2 = nc.dram_tensor("TH2", [H, T], f32, kind="ExternalInput")
    cur3 = nc.dram_tensor("cur3", [T, O, BL], f32, kind="ExternalOutput")

    with TileContext(nc) as tc:
        with tc.tile_pool(name="wpool", bufs=1) as wp:
            w1sb = wp.tile([DCH, NK * H], f32)
            nc.sync.dma_start(out=w1sb[:], in_=w1p[:])
            b1sb = wp.tile([H, 1], f32)
            nc.sync.dma_start(out=b1sb[:], in_=b1c[:])
            wsb = {}
            for nm, dram, dt in [
                ("W2a", W2a, bf16), ("W2b", W2b, bf16),
                ("D1a", D1a, f16), ("D1b", D1b, f16),
                ("D2a", D2a, f16), ("D2b", D2b, f16),
                ("W3a", W3a, bf16), ("W3b", W3b, bf16),
            ]:
                wsb[nm] = wp.tile(list(dram.shape), dt, name="wsb_" + nm)
                nc.sync.dma_start(out=wsb[nm][:], in_=dram[:])
            th2sb = wp.tile([H, T], f32)
            nc.sync.dma_start(out=th2sb[:], in_=TH2[:])
            cur1 = wp.tile([H, BL], f32)

            # ---- Phase 1: cur1 = w1 @ xT + b1, fp32, two column halves ----
            def _phase1():
              with (
                tc.tile_pool(name="fc1x", bufs=3) as xp,
                tc.tile_pool(name="fc1ps", bufs=1, space="PSUM") as pp1,
              ):
                for h in range(2):
                    ps = pp1.tile([H, FH], f32, tag="fc1")
                    for k in range(NK):
                        xt = xp.tile([DCH, FH], f32, tag="xt")
                        nc.gpsimd.dma_start(
                            out=xt[:],
                            in_=xT[k * DCH:(k + 1) * DCH, h * FH:(h + 1) * FH],
                        )
                        for s in range(FH // 512):
                            nc.tensor.matmul(
                                ps[:, s * 512:(s + 1) * 512],
                                lhsT=w1sb[:, k * H:(k + 1) * H],
                                rhs=xt[:, s * 512:(s + 1) * 512],
                                start=(k == 0),
                                stop=(k == NK - 1),
                            )
                    nc.scalar.activation(
                        cur1[:, h * FH:(h + 1) * FH], ps[:],
                        AF.Identity, bias=b1sb[:], scale=1.0,
                    )

            # ---- Phase 2: the 25-step scan, per column chunk ----
            # Software-pipelined emission: C2(t+1)/s1(t+1) are emitted before
            # E1(t)/E2(t) so the PE stream has filler work while ACT computes
            # the step-t layer-2 spikes. s2 is split into column halves so E1
            # can start as soon as the first half is ready.
            def _phase2():
              with (
                tc.tile_pool(name="spk", bufs=spk_bufs) as sp,
                tc.tile_pool(name="rp", bufs=3) as rp,
                tc.tile_pool(name="ps2", bufs=psum_bufs, space="PSUM") as pp2,
                tc.tile_pool(name="t3ps", bufs=2, space="PSUM") as pp3,
                tc.tile_pool(name="st3", bufs=3) as st3p,
              ):
                for c in range(nchunk):
                    c0 = c * bc
                    S1 = pp2.tile([H, bc], f32, tag="S1")
                    n2 = pp2.tile([H, bc], f32, tag="n2")
                    spk1s = {}
                    spk2ss = {}
                    t3s = {}

                    def emit_R_s1(t):
                        rt = rp.tile([H, bc], f32, tag="rt", name=f"rt{t}")
                        nc.vector.tensor_scalar(
                            rt[:], cur1[:, c0:c0 + bc],
                            _csum(t), -float(_binv(t)), ALU.mult, ALU.add,
                        )
                        spk1 = sp.tile([H, bc], bf16, tag="spk1",
                                       name=f"spk1_{t}")
                        spk1s[t] = spk1
                        if t == 0:
                            nc.vector.tensor_scalar(
                                spk1[:], rt[:], 0.0, None, ALU.is_gt,
                            )
                        else:
                            nc.vector.tensor_tensor(
                                spk1[:], rt[:], S1[:], ALU.is_gt,
                            )

                    def emit_C2(t):
                        if t >= T - 1:
                            return
                        spk1 = spk1s[t]
                        for s in range(nsl):
                            ssl = slice(s * 512, (s + 1) * 512)
                            nc.tensor.matmul(
                                S1[:, ssl], lhsT=wsb["D1a"][:, t * H:(t + 1) * H],
                                rhs=spk1[:, ssl],
                                start=(t == 0), stop=False,
                            )
                            nc.tensor.matmul(
                                S1[:, ssl], lhsT=wsb["D1b"][:, t * H:(t + 1) * H],
                                rhs=spk1[:, ssl],
                                start=False, stop=(t == T - 2),
                            )

                    def emit_C1(t):
                        spk1 = spk1s.pop(t)
                        for s in range(nsl):
                            ssl = slice(s * 512, (s + 1) * 512)
                            nc.tensor.matmul(
                                n2[:, ssl], lhsT=wsb["W2a"][:, t * H:(t + 1) * H],
                                rhs=spk1[:, ssl],
                                start=(t == 0), stop=False,
                            )
                            nc.tensor.matmul(
                                n2[:, ssl], lhsT=wsb["W2b"][:, t * H:(t + 1) * H],
                                rhs=spk1[:, ssl],
                                start=False, stop=(t == T - 1),
                            )

                    def emit_s2(t):
                        spk2s = sp.tile([H, bc], bf16, tag="spk2s",
                                        name=f"spk2s_{t}")
                        spk2ss[t] = spk2s
                        hw = bc // 2
                        for hh in range(2):
                            hsl = slice(hh * hw, (hh + 1) * hw)
                            nc.scalar.activation(
                                spk2s[:, hsl], n2[:, hsl], AF.Sign,
                                bias=th2sb[:, t:t + 1], scale=1.0,
                            )

                    def emit_E1(t):
                        if t >= T - 1:
                            return
                        spk2s = spk2ss[t]
                        for s in range(nsl):
                            ssl = slice(s * 512, (s + 1) * 512)
                            for p in ("D2a", "D2b"):
                                nc.tensor.matmul(
                                    n2[:, ssl], lhsT=wsb[p][:, t * H:(t + 1) * H],
                                    rhs=spk2s[:, ssl],
                                    start=False, stop=False,
                                )

                    def emit_E2(t):
                        j = t % 4
                        if j == 0:
                            t3s[t // 4] = pp3.tile([128, bc], f32, tag="t3",
                                                   name=f"t3_{t}")
                        t3 = t3s[t // 4]
                        spk2s = spk2ss.pop(t)
                        for s in range(nsl):
                            ssl = slice(s * 512, (s + 1) * 512)
                            nc.tensor.matmul(
                                t3[32 * j:32 * j + O, ssl],
                                lhsT=wsb["W3a"][:, t * O:(t + 1) * O],
                                rhs=spk2s[:, ssl],
                                start=True, stop=False,
                                tile_position=(0, 32 * j),
                            )
                            nc.tensor.matmul(
                                t3[32 * j:32 * j + O, ssl],
                                lhsT=wsb["W3b"][:, t * O:(t + 1) * O],
                                rhs=spk2s[:, ssl],
                                start=False, stop=True,
                                tile_position=(0, 32 * j),
                            )
                        if j == 3 or t == T - 1:
                            g = t // 4
                            t3 = t3s.pop(g)
                            stage = st3p.tile([128, bc], f32, tag="stage",
                                              name=f"stage{t}")
                            nc.scalar.activation(stage[:], t3[:], AF.Copy)
                            for jj in range(j + 1):
                                tt = 4 * g + jj
                                nc.sync.dma_start(
                                    out=cur3[tt, :, c0:c0 + bc],
                                    in_=stage[32 * jj:32 * jj + O, :],
                                )

                    emit_R_s1(0)
                    emit_C2(0)
                    emit_C1(0)
                    for t in range(T):
                        if t + 1 < T:
                            emit_R_s1(t + 1)
                            emit_C2(t + 1)
                        emit_s2(t)
                        emit_E1(t)
                        emit_E2(t)
                        if t + 1 < T:
                            emit_C1(t + 1)

            for _rep in range(repeat):
                if do_fc1:
                    _phase1()
                if do_scan:
                    _phase2()

    # Walrus codegen allows at most one semaphore wait per instruction;
    # split/migrate excess waits the same way Bacc.compile() does.
    import bass_rust as _bass_rust
    _bass_rust.move_matmul_waits_to_ldweights(nc.m)
    _bass_rust.generate_event_semaphores(nc)
    return nc


def _get_program(repeat=1):
    key = f"nc{repeat}"
    if key not in _CACHE:
        _CACHE[key] = _build_program(repeat)
    return _CACHE[key]


def _split2(M, dt):
    hi = M.astype(dt)
    lo = (M - hi.astype(np.float32)).astype(dt)
    return hi, lo


def _host_weights(w1, b1, w2, b2, w3, b3):
    import ml_dtypes
    bf = ml_dtypes.bfloat16
    f16 = np.float16
    f64 = np.float64

    w1T = np.ascontiguousarray(w1.T)                       # [784, 100]
    w1p = np.empty((DCH, NK * H), np.float32)
    for k in range(NK):
        w1p[:, k * H:(k + 1) * H] = w1T[k * DCH:(k + 1) * DCH]
    b1c = np.ascontiguousarray(b1.reshape(H, 1))

    W2f = np.zeros((H, T * H), np.float32)
    D1f = np.zeros((H, T * H), np.float32)
    D2f = np.zeros((H, T * H), np.float32)
    W3f = np.zeros((H, T * O), np.float32)
    TH2 = np.empty((H, T), np.float32)
    idx = np.arange(H)
    for t in range(T):
        bt = f64(BETA) ** (-t)
        btn = f64(BETA) ** (-(t + 1))
        ct = _csum(t)
        W2f[:, t * H:(t + 1) * H] = (bt * w2.astype(f64)).T.astype(np.float32)
        D1f[idx, t * H + idx] = np.float32(btn)
        D2f[idx, t * H + idx] = np.float32(-0.5 * btn)
        W3f[:, t * O:(t + 1) * O] = (0.5 * w3.astype(f64)).T.astype(np.float32)
        # ACT Sign bias: spk2 test is  n2_dev > theta2 = beta^-t - c_t*b2
        #   + 0.5*(c_t - 1); bias = -theta2
        TH2[:, t] = (-(bt - ct * b2.astype(f64) + 0.5 * (ct - 1))).astype(
            np.float32)
    out = dict(w1p=w1p, b1c=b1c, TH2=TH2)
    out["W2a"], out["W2b"] = _split2(W2f, bf)
    out["D1a"], out["D1b"] = _split2(D1f, f16)
    out["D2a"], out["D2b"] = _split2(D2f, f16)
    out["W3a"], out["W3b"] = _split2(W3f, bf)
    return out


def _host_lif3(cur3_dev, w3, b3):
    """cur3_dev: [T, B, 2] fp32 device part -> (spk3_rec, mem3_rec)."""
    f32 = np.float32
    c3 = (b3.astype(np.float64)
          + 0.5 * w3.astype(np.float64).sum(axis=1)).astype(f32)
    n = cur3_dev.shape[1]
    m3 = np.zeros((n, O), f32)
    spk3 = np.zeros((T, n, O), f32)
    mem3 = np.zeros((T, n, O), f32)
    rst = np.zeros((n, O), f32)
    beta = f32(BETA)
    th = f32(TH)
    for t in range(T):
        m3 = beta * m3 + (cur3_dev[t] + c3[None, :]) - rst * th
        s = ((m3 - th) > 0).astype(f32)
        spk3[t] = s
        mem3[t] = m3
        rst = s
    return spk3, mem3


def kernel(x, w1, b1, w2, b2, w3, b3):
    from concourse.bass_utils import run_bass_kernel_spmd

    x = np.asarray(x, np.float32)
    w3 = np.asarray(w3, np.float32)
    b3 = np.asarray(b3, np.float32)
    shared = _host_weights(
        np.asarray(w1, np.float32), np.asarray(b1, np.float32),
        np.asarray(w2, np.float32), np.asarray(b2, np.float32),
        w3, b3,
    )
    in_maps = []
    for i in range(NCORES):
        xi = np.ascontiguousarray(x[i * BL:(i + 1) * BL].T)  # [784, 8192]
        m = dict(shared)
        m["xT"] = xi
        in_maps.append(m)

    nc = _get_program()
    res = run_bass_kernel_spmd(nc, in_maps, core_ids=list(range(NCORES)))

    cur3_dev = np.empty((T, B, O), np.float32)
    for i in range(NCORES):
        ci = res.results[i]["cur3"]          # [T, 2, BL]
        cur3_dev[:, i * BL:(i + 1) * BL, :] = np.transpose(ci, (0, 2, 1))

    return _host_lif3(cur3_dev, w3, b3)
